# revision 1
# baseline (speedup 1.0000x reference)
"""BYOL loss kernel for Trainium2 (8 NeuronCores, SPMD data-parallel).

loss = 2 - 2 * mean_n( <x_n, t_n> / (||x_n|| * ||t_n||) )   over N=8192 rows, D=512.

Sharding: rows split 1024/core across 8 cores. Each core computes the per-row
cosine for its shard (DVE fused multiply-reduce for the dot, ScalarE
square-with-accumulate for the norms); host gathers the 8192 per-row cosines
and takes the mean (the trivial "all-reduce" step).
"""

import sys

for _p in ("/opt/trn_rl_repo",):
    if _p not in sys.path:
        sys.path.insert(0, _p)

import numpy as np

import concourse.tile as tile
from concourse import bacc, mybir
from concourse import bass_utils

N, D = 8192, 512
NCORES = 8
N_LOC = N // NCORES          # 1024 rows per core
P = 128                      # partitions
NT = N_LOC // P              # 8 row-blocks of [128, 512] per core
CHUNKS = [1, 1, 2, 2, 1, 1]  # row-blocks per dma_start (small first = early start,
                             # small last = short pipeline drain)
IN_BUFS = len(CHUNKS)        # all chunks in flight: DMA ring never starves
SQUARE_ON_DVE = {(1, 2)}     # (tensor_idx, block) square-sums moved ACT -> DVE
                             # to balance engines (ACT 15 ops ~12.0us, DVE ~11.6us)
USE_BF16 = False             # cast inputs f32->bf16 during DMA (SWDGE): DVE muls 2x
                             # but ACT squares don't speed up and SWDGE adds ~2.5us
                             # startup latency — measured slower than HWDGE f32
T_ON_ACT_RING = False        # t-loads from ACT sequencer (qActDynamicHW): crashes
                             # the device in this stack — keep False

F32 = mybir.dt.float32
BF16 = mybir.dt.bfloat16


def _build():
    nc = bacc.Bacc("TRN2", target_bir_lowering=False, debug=False, num_devices=NCORES)
    x = nc.dram_tensor("x", [N_LOC, D], F32, kind="ExternalInput").ap()
    t = nc.dram_tensor("t", [N_LOC, D], F32, kind="ExternalInput").ap()
    # per-row dots: [:, 0:NT] = <x,t>, [:, NT:2NT] = <x,x>, [:, 2NT:3NT] = <t,t>
    out = nc.dram_tensor("dots", [P, 3 * NT], F32, kind="ExternalOutput").ap()

    # row r = i*128 + p  ->  tile index i, partition p
    xr = x.rearrange("(t p) d -> p t d", p=P)
    tr = t.rearrange("(t p) d -> p t d", p=P)

    assert sum(CHUNKS) == NT

    with tile.TileContext(nc) as tc:
        with (
            tc.tile_pool(name="xin", bufs=IN_BUFS) as xpool,
            tc.tile_pool(name="tin", bufs=IN_BUFS) as tpool,
            tc.tile_pool(name="scratch", bufs=4) as spool,
            tc.tile_pool(name="stats", bufs=1) as stats,
        ):
            dots = stats.tile([P, 3 * NT], F32, tag="dots")
            xt_s = dots[:, 0:NT]
            xx_s = dots[:, NT : 2 * NT]
            tt_s = dots[:, 2 * NT : 3 * NT]

            in_dt = BF16 if USE_BF16 else F32
            # t-loads on the second HWDGE ring (ACT sequencer issues them):
            # two FIFO rings interleave across the 16 SDMA engines
            t_dma_engine = nc.scalar if T_ON_ACT_RING else nc.sync

            def square_sum(tensor_idx, block, src_ap, acc_ap):
                if (tensor_idx, block) in SQUARE_ON_DVE:
                    sq = spool.tile([P, D], in_dt, tag="prod")
                    nc.vector.tensor_mul(sq[:], src_ap, src_ap)
                    nc.vector.reduce_sum(acc_ap, sq[:], axis=mybir.AxisListType.X)
                else:
                    sq = spool.tile([P, D], in_dt, tag="sq")
                    nc.scalar.activation(
                        sq[:], src_ap, mybir.ActivationFunctionType.Square,
                        accum_out=acc_ap,
                    )

            base = 0
            for sz in CHUNKS:
                xt_in = xpool.tile([P, sz, D], in_dt, tag="xd")
                tt_in = tpool.tile([P, sz, D], in_dt, tag="td")
                if USE_BF16:
                    nc.gpsimd.dma_start(xt_in[:], xr[:, base : base + sz, :])
                    nc.gpsimd.dma_start(tt_in[:], tr[:, base : base + sz, :])
                else:
                    nc.sync.dma_start(xt_in[:], xr[:, base : base + sz, :])
                    t_dma_engine.dma_start(tt_in[:], tr[:, base : base + sz, :])
                for j in range(sz):
                    i = base + j
                    xa = xt_in[:, j, :]
                    ta = tt_in[:, j, :]
                    square_sum(0, i, xa, xx_s[:, i : i + 1])
                    square_sum(1, i, ta, tt_s[:, i : i + 1])
                # <x, t>: one VectorE multiply + one free-dim reduce per chunk
                prod = spool.tile([P, sz, D], in_dt, tag="prod")
                nc.vector.tensor_mul(prod[:], xt_in[:], tt_in[:])
                nc.vector.reduce_sum(
                    xt_s[:, base : base + sz], prod[:], axis=mybir.AxisListType.X
                )
                base += sz

            nc.sync.dma_start(out, dots[:])

    nc.finalize()
    return nc


_nc_cache = None


def _get_nc():
    global _nc_cache
    if _nc_cache is None:
        _nc_cache = _build()
    return _nc_cache


def run(x, x_target, **spmd_kwargs):
    """Run the SPMD kernel; returns (loss, BassKernelResults)."""
    x = np.ascontiguousarray(np.asarray(x, dtype=np.float32))
    t = np.ascontiguousarray(np.asarray(x_target, dtype=np.float32))
    assert x.shape == (N, D) and t.shape == (N, D)
    nc = _get_nc()
    in_maps = [
        {
            "x": x[c * N_LOC : (c + 1) * N_LOC],
            "t": t[c * N_LOC : (c + 1) * N_LOC],
        }
        for c in range(NCORES)
    ]
    res = bass_utils.run_bass_kernel_spmd(
        nc, in_maps, core_ids=list(range(NCORES)), **spmd_kwargs
    )
    dots = np.stack([np.asarray(r["dots"]) for r in res.results]).astype(np.float64)
    xt = dots[:, :, 0:NT]
    xx = dots[:, :, NT : 2 * NT]
    tt = dots[:, :, 2 * NT : 3 * NT]
    EPS = 1e-8  # matches reference: a / max(||a||, eps) per tensor
    cos = xt / (np.maximum(np.sqrt(xx), EPS) * np.maximum(np.sqrt(tt), EPS))
    loss = 2.0 - 2.0 * float(np.mean(cos))
    return np.float32(loss), res


def kernel(x, x_target):
    loss, _ = run(x, x_target)
    return loss



# revision 5
# speedup vs baseline: 1.1442x; 1.1442x over previous
"""BYOL loss kernel for Trainium2 (8 NeuronCores, SPMD data-parallel).

loss = 2 - 2 * mean_n( <x_n, t_n> / (||x_n|| * ||t_n||) )   over N=8192 rows, D=512.

v3 design (HW-measured op costs):
- Host casts f32 -> bf16 and reshapes each core's [1024, 512] shard to
  [128, 4096]: row r = p*8 + slot lives on partition p. Every DMA is one
  contiguous run per partition (128 descriptors/dma_start) and total HBM
  traffic halves to 2 MiB/core vs f32.
- Per-row products via DVE scalar_tensor_tensor (fused multiply + accum,
  687 ns/block measured; tensor_tensor_reduce crashes this stack) and ACT
  Square-with-accumulate (~1157 ns/block incl READ_ACC). 24 block-products
  split ACT:10 / DVE:14 to balance both engines (~10.5 us each), overlapped
  over the ~7 us DMA stream.
- Host gathers the [128, 24] f32 per-core stats and finishes cosine + mean
  in f64 (the trivial all-reduce).
"""

import sys

for _p in ("/opt/trn_rl_repo",):
    if _p not in sys.path:
        sys.path.insert(0, _p)

import ml_dtypes
import numpy as np

import concourse.tile as tile
from concourse import bacc, mybir
from concourse import bass_utils

N, D = 8192, 512
NCORES = 8
N_LOC = N // NCORES          # 1024 rows per core
P = 128                      # partitions
NT = N_LOC // P              # 8 blocks of [128, 512] per core
CHUNKS = [1, 2, 2, 3]        # blocks per dma_start per tensor; small first chunk
                             # so ACT/DVE start early
# products: ("xx"|"tt"|"xt", block) -> engine. ACT takes 10 (early blocks),
# DVE (STT) takes 14.
ACT_PRODUCTS = {("xx", i) for i in range(5)} | {("tt", i) for i in range(5)}

F32 = mybir.dt.float32
BF16 = mybir.dt.bfloat16
MULT = mybir.AluOpType.mult


def _build():
    nc = bacc.Bacc("TRN2", target_bir_lowering=False, debug=False, num_devices=NCORES)
    x = nc.dram_tensor("x", [P, NT * D], BF16, kind="ExternalInput").ap()
    t = nc.dram_tensor("t", [P, NT * D], BF16, kind="ExternalInput").ap()
    # per-row stats: [:, 0:NT] = <x,t>, [:, NT:2NT] = <x,x>, [:, 2NT:3NT] = <t,t>
    out = nc.dram_tensor("dots", [P, 3 * NT], F32, kind="ExternalOutput").ap()
    col = {"xt": 0, "xx": NT, "tt": 2 * NT}

    assert sum(CHUNKS) == NT

    with tile.TileContext(nc) as tc:
        with (
            tc.tile_pool(name="xin", bufs=len(CHUNKS)) as xpool,
            tc.tile_pool(name="tin", bufs=len(CHUNKS)) as tpool,
            tc.tile_pool(name="scratch", bufs=3) as spool,
            tc.tile_pool(name="stats", bufs=1) as stats,
        ):
            dots = stats.tile([P, 3 * NT], F32, tag="dots")
            warm = stats.tile([P, 1], BF16, tag="warm")
            # tiny first ACT op: ACT_TABLE_LOAD overlaps the DMA wait
            one_bf16 = nc.const_aps.aps[(BF16, 1.0)]
            nc.scalar.activation(warm[:], one_bf16, mybir.ActivationFunctionType.Square)

            def acc_ap(stat, i):
                return dots[:, col[stat] + i : col[stat] + i + 1]

            def act_square(src, stat, i):
                sq = spool.tile([P, D], BF16, tag="sq")
                nc.scalar.activation(
                    sq[:], src, mybir.ActivationFunctionType.Square,
                    accum_out=acc_ap(stat, i),
                )

            def dve_stt(a, b, stat, i):
                pr = spool.tile([P, D], BF16, tag="pr")
                nc.vector.scalar_tensor_tensor(
                    pr[:], a, 1.0, b, op0=MULT, op1=MULT,
                    accum_out=acc_ap(stat, i),
                )

            base = 0
            xblk, tblk = {}, {}
            for sz in CHUNKS:
                xin = xpool.tile([P, sz * D], BF16, tag="xd")
                tin = tpool.tile([P, sz * D], BF16, tag="td")
                nc.sync.dma_start(xin[:], x[:, base * D : (base + sz) * D])
                nc.sync.dma_start(tin[:], t[:, base * D : (base + sz) * D])
                for j in range(sz):
                    i = base + j
                    xblk[i] = xin[:, j * D : (j + 1) * D]
                    tblk[i] = tin[:, j * D : (j + 1) * D]
                # ACT first (starts on x-chunk alone), then DVE squares that
                # depend only on this x-chunk, then the xt/tt products.
                for j in range(sz):
                    i = base + j
                    if ("xx", i) in ACT_PRODUCTS:
                        act_square(xblk[i], "xx", i)
                    else:
                        dve_stt(xblk[i], xblk[i], "xx", i)
                for j in range(sz):
                    i = base + j
                    if ("tt", i) in ACT_PRODUCTS:
                        act_square(tblk[i], "tt", i)
                    else:
                        dve_stt(tblk[i], tblk[i], "tt", i)
                    dve_stt(xblk[i], tblk[i], "xt", i)
                base += sz

            nc.sync.dma_start(out, dots[:])

    nc.finalize()
    return nc


_nc_cache = None


def _get_nc():
    global _nc_cache
    if _nc_cache is None:
        _nc_cache = _build()
    return _nc_cache


def run(x, x_target, **spmd_kwargs):
    """Run the SPMD kernel; returns (loss, BassKernelResults)."""
    x = np.asarray(x, dtype=np.float32).astype(ml_dtypes.bfloat16)
    t = np.asarray(x_target, dtype=np.float32).astype(ml_dtypes.bfloat16)
    assert x.shape == (N, D) and t.shape == (N, D)
    nc = _get_nc()
    in_maps = [
        {
            # row r = p*NT + slot: plain C-order reshape of the shard
            "x": np.ascontiguousarray(x[c * N_LOC : (c + 1) * N_LOC]).reshape(P, NT * D),
            "t": np.ascontiguousarray(t[c * N_LOC : (c + 1) * N_LOC]).reshape(P, NT * D),
        }
        for c in range(NCORES)
    ]
    res = bass_utils.run_bass_kernel_spmd(
        nc, in_maps, core_ids=list(range(NCORES)), **spmd_kwargs
    )
    dots = np.stack([np.asarray(r["dots"]) for r in res.results]).astype(np.float64)
    xt = dots[:, :, 0:NT]
    xx = dots[:, :, NT : 2 * NT]
    tt = dots[:, :, 2 * NT : 3 * NT]
    EPS = 1e-8  # matches reference: a / max(||a||, eps) per tensor
    cos = xt / (np.maximum(np.sqrt(xx), EPS) * np.maximum(np.sqrt(tt), EPS))
    loss = 2.0 - 2.0 * float(np.mean(cos))
    return np.float32(loss), res


def kernel(x, x_target):
    loss, _ = run(x, x_target)
    return loss
